# revision 26
# baseline (speedup 1.0000x reference)
"""SAGAN-style self-attention block on 8 Trainium2 NeuronCores (v2).

Reference computation (per batch element b, C=128, H=W=64, N=4096, M=1024):
    theta = W_theta @ x_b                       [16, 4096]
    phi   = maxpool2x2(W_phi @ x_b)             [16, 1024]
    g     = maxpool2x2(W_g @ x_b)               [64, 1024]
    S     = theta^T phi                         [4096, 1024]
    beta  = softmax(S, axis=-1)
    o     = g @ beta^T                          [64, 4096]
    out   = gamma * (W_o @ o) + x_b             [128, 4096]

Sharding: data-parallel over batch; core b gets batch element b; weights
replicated; no collectives.

v2 design notes (engine budgets from the TimelineSim cost model; op cost
scales with free-dim columns only, GPSIMD/Pool cannot touch PSUM):
  - bf16 on all matmul paths (1 cyc/row PE streaming like fp32r, half the
    DMA), f32 psum, bf16 residual (total rel err ~2e-3 vs 2e-2 gate).
    gamma folded into W_o on the host.
  - Act engine is the hard floor: exp = 32 x [128,1024] tiles ~32us and
    only the scalar engine has Exp, so Act does exp and nothing else.
  - conv A: stationary [128K, 80M] = [W_g^T | W_phi^T] -> psum rows 0:64
    g, 64:80 phi; ONE fused maxpool tensor_reduce drains both (DVE).
    conv B: stationary [128K, 16M] = W_theta^T with tile_position=(0,64)
    so theta lands on partition band 64 = phi's band: the K=16 S^T
    matmul needs stationary phi and moving theta on one 32-aligned band.
  - S^T chunk [128m, 1024n] = phi_mi^T theta (32 matmuls). exp -> et
    bf16. po[128,512] accumulates gTa_mi^T et_mi with the ones-column
    trick (row 0 = softmax denominator, rows 64:128 = unnorm o). The po
    matmuls trail one mi behind the S^T matmuls in the PE stream so the
    PE fills its exp-wait gaps; the two chunks per superchunk accumulate
    in parallel psum banks.
  - Epilogue per 512-chunk: s->bf16 copy (DVE), broadcast via ones
    matmul (PE), reciprocal_approx_fast (DVE), o*(1/s) fused into the
    psum->sbuf drain (DVE), po2 = (gamma W_o) @ o_norm (PE),
    out = po2 + x (DVE), DMA per 1024 cols.
  - PSUM exactly 8 banks: pst 2x[128,1024] (4) + psm 4x[*,512] (4)
    shared by convs / po accumulators / pbc / po2 in rotation.
"""

import numpy as np

N_CORES = 8
C = 128
N = 4096       # H*W
M = 1024       # N/4
NCH = 8        # 512-col chunks
CHUNK = 512


def _build(gamma: float, reps: int = 1):
    from contextlib import nullcontext
    import concourse.bass as bass
    import concourse.tile as tile
    from concourse import bacc, mybir

    f32 = mybir.dt.float32
    bf16 = mybir.dt.bfloat16
    ts = bass.ts
    ALU = mybir.AluOpType
    ACTF = mybir.ActivationFunctionType

    nc = bacc.Bacc(
        "TRN2", target_bir_lowering=False, debug=False, enable_asserts=False,
        num_devices=N_CORES,
    )
    xb_d = nc.dram_tensor("xb", [C, N], bf16, kind="ExternalInput")
    # cols 0:64 W_g^T, 64:80 W_phi^T, 80:96 W_theta^T,
    # 96:224 (gamma*W_o)^T on rows 64:128
    wcat_d = nc.dram_tensor("wcat", [128, 224], bf16, kind="ExternalInput")
    id_d = nc.dram_tensor("ident", [64, 64], bf16, kind="ExternalInput")
    out_d = nc.dram_tensor("out", [C, N], f32, kind="ExternalOutput")

    with tile.TileContext(nc) as tc:
        with (
            tc.tile_pool(name="persist", bufs=1) as persist,
            tc.tile_pool(name="et", bufs=10) as etp,
            tc.tile_pool(name="osb", bufs=3) as osbp,
            tc.tile_pool(name="rcb", bufs=3) as rcbp,
            tc.tile_pool(name="ssb", bufs=3) as ssbp,
            tc.tile_pool(name="outp", bufs=3) as outpool,
            tc.tile_pool(name="pst", bufs=2, space="PSUM") as pst,
            tc.tile_pool(name="psm", bufs=4, space="PSUM") as psm,
        ):
          loop_cm = (
              tc.For_i(
                  0, reps, 1,
                  hint_engines=(
                      mybir.EngineType.PE,
                      mybir.EngineType.DVE,
                      mybir.EngineType.Activation,
                      mybir.EngineType.SP,
                      mybir.EngineType.Pool,
                  ),
              )
              if reps > 1
              else nullcontext()
          )
          with loop_cm:
            # ---- loads: wcat on the Act HWDGE queue (parallel with xb on
            # SP); first xb pieces small so conv(0)/conv(1) start early ---
            wcat = persist.tile([128, 224], bf16, name="wcat")
            nc.scalar.dma_start(wcat, wcat_d[:, :])
            xb = persist.tile([C, N], bf16, name="xb")
            for lo, sz in ((0, 512), (512, 512), (1024, 1024),
                           (2048, 1024), (3072, 1024)):
                nc.sync.dma_start(xb[:, bass.ds(lo, sz)],
                                  xb_d[:, bass.ds(lo, sz)])
            id_sb = persist.tile([64, 64], bf16, name="id_sb")
            nc.sync.dma_start(id_sb, id_d[:, :])

            wA = wcat[:, 0:80]          # [W_g^T | W_phi^T]
            wB = wcat[:, 80:96]         # W_theta^T
            wo = wcat[64:128, 96:224]   # (gamma*W_o)^T, K band 64

            ones_sb = persist.tile([1, 128], bf16, name="ones_sb")
            nc.vector.memset(ones_sb, 1.0)

            theta_sb = persist.tile([80, N], bf16, name="theta_sb")  # rows 64:80
            # rows 0:64 g, rows 64:80 phi
            phig_sb = persist.tile([80, M], bf16, name="phig_sb")
            gTa = persist.tile([128, 8 * 128], bf16, name="gTa")
            nc.gpsimd.memset(gTa, 0.0)
            for mi in range(8):
                nc.vector.memset(gTa[:, mi * 128 : mi * 128 + 1], 1.0)

            # ---- per-chunk conv emission -------------------------------
            def conv(ci):
                xc = xb[:, ts(ci, CHUNK)]
                psA = psm.tile([80, CHUNK], f32, name="psA", tag="sm")
                nc.tensor.matmul(psA, wA, xc, start=True, stop=True,
                                 tile_position=(0, 0))
                psB = psm.tile([128, CHUNK], f32, name="psB", tag="sm")
                nc.tensor.matmul(psB[64:80, :], wB, xc, start=True, stop=True,
                                 tile_position=(0, 64))
                # fused g+phi maxpool (DVE) — phi gates S^T(0, ci)
                nc.vector.tensor_reduce(
                    out=phig_sb[:, ts(ci, 128)].rearrange(
                        "p (i j) -> p i j", i=4, j=32),
                    in_=psA.rearrange(
                        "p (i di j dj) -> p i j di dj", i=4, di=2, j=32, dj=2),
                    axis=mybir.AxisListType.XY,
                    op=ALU.max,
                )
                # theta copy: Act during ramp (idle pre-exp; Copy shares the
                # Exp activation table so no reload), DVE after
                dst = theta_sb[64:80, ts(ci, CHUNK)]
                if ci < 2:
                    nc.scalar.copy(dst, psB[64:80, :])
                else:
                    nc.vector.tensor_copy(dst, psB[64:80, :])

            ets = {}

            def emit_st(k, mi):
                st = pst.tile([128, 1024], f32, name="st", tag="st")
                for h in range(2):
                    nc.tensor.matmul(
                        st[:, ts(h, CHUNK)], phig_sb[64:80, ts(mi, 128)],
                        theta_sb[64:80, bass.ds(k * 1024 + h * CHUNK, CHUNK)],
                        start=True, stop=True, tile_position=(64, 0))
                et = etp.tile([128, 1024], bf16, name="et", tag="et")
                nc.scalar.activation(et, st, ACTF.Exp)
                ets[(k, mi)] = et

            def transp(mi):
                ptr = psm.tile([128, 64], bf16, name="ptr", tag="sm")
                nc.tensor.transpose(ptr, phig_sb[0:64, ts(mi, 128)], id_sb)
                nc.vector.tensor_copy(gTa[:, mi * 128 + 64 : mi * 128 + 128],
                                      ptr)

            # ---- attention (v2 schedule: tight S^T/exp/po interleave;
            # the PE stream never queues long runs of blocked matmuls —
            # HW measures ~20% slower when it does, despite the cost
            # model predicting the opposite) ----------------------------
            # per chunk epilogue: s->bf16 (DVE), broadcast via ones matmul
            # (PE), recip full-tile (DVE), o*(1/s) psum drain (DVE),
            # po2 (PE), residual add (DVE), DMA. GPSIMD avoided: ~2us
            # fixed overhead per op on HW.
            def epilogue(k, pos):
                ss, rbs, os = [], [], []
                for c01 in range(2):
                    s_sb = ssbp.tile([1, CHUNK], bf16, name="s_sb", tag="s")
                    nc.vector.tensor_copy(s_sb, pos[c01][0:1, :])
                    ss.append(s_sb)
                # k=3 epilogue psum comes from the pst pool (free after the
                # last exp) so the next iteration's convs are not gated on
                # this iteration's tail via psm slot reuse
                eppool, eptag = (pst, "st") if k == 3 else (psm, "sm")
                pbcs = []
                for c01 in range(2):
                    pbc = eppool.tile([128, CHUNK], f32, name="pbc", tag=eptag)
                    nc.tensor.matmul(pbc, ones_sb, ss[c01], start=True,
                                     stop=True, tile_position=(0, 0))
                    pbcs.append(pbc)
                for c01 in range(2):
                    rbc = rcbp.tile([128, CHUNK], f32, name="rbc", tag="r")
                    nc.vector.reciprocal_approx_fast(rbc, pbcs[c01])
                    rbs.append(rbc)
                for c01 in range(2):
                    o_sb = osbp.tile([128, CHUNK], bf16, name="o_sb", tag="o")
                    nc.vector.tensor_tensor(o_sb[64:128, :],
                                            pos[c01][64:128, :],
                                            rbs[c01][64:128, :], op=ALU.mult)
                    os.append(o_sb)
                po2s = []
                for c01 in range(2):
                    po2 = eppool.tile([128, CHUNK], f32, name="po2", tag=eptag)
                    nc.tensor.matmul(po2, wo, os[c01][64:128, :],
                                     start=True, stop=True,
                                     tile_position=(64, 0))
                    po2s.append(po2)
                outp = outpool.tile([128, 2 * CHUNK], f32, name="outp",
                                    tag="out")
                for c01 in range(2):
                    nc.vector.tensor_tensor(outp[:, ts(c01, CHUNK)],
                                            po2s[c01],
                                            xb[:, ts(2 * k + c01, CHUNK)],
                                            op=ALU.add)
                # out via the Pool SWDGE queue: keeps the SP HWDGE queue
                # free so the next iteration's xb loads dispatch early
                nc.gpsimd.dma_start(out_d[:, bass.ds(2 * k * CHUNK, 2 * CHUNK)],
                                    outp)

            # k=0 interleaves the convs into the S^T stream (the ramp);
            # transposes batch after conv(7) — they are ready-to-run by
            # then, and the 4 psm slots are fully booked by in-flight
            # convs + the po accumulators, so ptr tiles must not overlap
            # the convs' slots lifetime.
            conv(0)
            conv(1)
            for k in range(4):
                if k == 0:
                    for mi in range(6):
                        emit_st(0, mi)
                        conv(mi + 2)
                    for mi in range(8):
                        transp(mi)

                po_a = psm.tile([128, CHUNK], f32, name="po_a", tag="sm")
                po_b = psm.tile([128, CHUNK], f32, name="po_b", tag="sm")
                pos = (po_a, po_b)

                def po_step(mi):
                    for c01 in range(2):
                        nc.tensor.matmul(
                            pos[c01], gTa[:, ts(mi, 128)],
                            ets[(k, mi)][:, ts(c01, CHUNK)],
                            start=(mi == 0), stop=(mi == 7),
                            tile_position=(0, 0))

                if k == 0:
                    emit_st(0, 6)
                    for mi in range(5):
                        po_step(mi)
                    emit_st(0, 7)
                    po_step(5)
                    po_step(6)
                else:
                    for mi in range(8):
                        emit_st(k, mi)
                        if mi >= 1:
                            po_step(mi - 1)
                po_step(7)
                epilogue(k, pos)

    nc.compile()
    return nc


def _host_prep(x, W_theta, W_phi, W_g, W_o, gamma=0.5):
    import ml_dtypes
    bf16 = ml_dtypes.bfloat16
    x = np.ascontiguousarray(np.asarray(x, dtype=np.float32))
    B = x.shape[0]
    wcat = np.zeros((128, 224), dtype=np.float32)
    wcat[:, 0:64] = np.asarray(W_g, np.float32).T
    wcat[:, 64:80] = np.asarray(W_phi, np.float32).T
    wcat[:, 80:96] = np.asarray(W_theta, np.float32).T
    wcat[64:128, 96:224] = (float(gamma) * np.asarray(W_o, np.float32)).T
    wcat = wcat.astype(bf16)
    ident = np.eye(64, dtype=np.float32).astype(bf16)
    in_maps = []
    for b in range(B):
        xb = np.ascontiguousarray(x[b].reshape(C, N)).astype(bf16)
        in_maps.append({"xb": xb, "wcat": wcat, "ident": ident})
    return in_maps


def run(x, W_theta, W_phi, W_g, W_o, gamma, trace=False, **trace_kwargs):
    from concourse.bass_utils import run_bass_kernel_spmd

    nc = _build(float(np.asarray(gamma)))
    in_maps = _host_prep(x, W_theta, W_phi, W_g, W_o, float(np.asarray(gamma)))
    res = run_bass_kernel_spmd(
        nc, in_maps, core_ids=list(range(N_CORES)), trace=trace, **trace_kwargs
    )
    outs = [res.results[b]["out"].reshape(C, 64, 64) for b in range(N_CORES)]
    return np.stack(outs).astype(np.float32), res


def kernel(x, W_theta, W_phi, W_g, W_o, gamma):
    out, _ = run(x, W_theta, W_phi, W_g, W_o, gamma)
    return out


# revision 27
# speedup vs baseline: 1.0095x; 1.0095x over previous
"""SAGAN-style self-attention block on 8 Trainium2 NeuronCores (v2).

Reference computation (per batch element b, C=128, H=W=64, N=4096, M=1024):
    theta = W_theta @ x_b                       [16, 4096]
    phi   = maxpool2x2(W_phi @ x_b)             [16, 1024]
    g     = maxpool2x2(W_g @ x_b)               [64, 1024]
    S     = theta^T phi                         [4096, 1024]
    beta  = softmax(S, axis=-1)
    o     = g @ beta^T                          [64, 4096]
    out   = gamma * (W_o @ o) + x_b             [128, 4096]

Sharding: data-parallel over batch; core b gets batch element b; weights
replicated; no collectives.

v2 design notes (engine budgets from the TimelineSim cost model; op cost
scales with free-dim columns only, GPSIMD/Pool cannot touch PSUM):
  - bf16 on all matmul paths (1 cyc/row PE streaming like fp32r, half the
    DMA), f32 psum, bf16 residual (total rel err ~2e-3 vs 2e-2 gate).
    gamma folded into W_o on the host.
  - Act engine is the hard floor: exp = 32 x [128,1024] tiles ~32us and
    only the scalar engine has Exp, so Act does exp and nothing else.
  - conv A: stationary [128K, 80M] = [W_g^T | W_phi^T] -> psum rows 0:64
    g, 64:80 phi; ONE fused maxpool tensor_reduce drains both (DVE).
    conv B: stationary [128K, 16M] = W_theta^T with tile_position=(0,64)
    so theta lands on partition band 64 = phi's band: the K=16 S^T
    matmul needs stationary phi and moving theta on one 32-aligned band.
  - S^T chunk [128m, 1024n] = phi_mi^T theta (32 matmuls). exp -> et
    bf16. po[128,512] accumulates gTa_mi^T et_mi with the ones-column
    trick (row 0 = softmax denominator, rows 64:128 = unnorm o). The po
    matmuls trail one mi behind the S^T matmuls in the PE stream so the
    PE fills its exp-wait gaps; the two chunks per superchunk accumulate
    in parallel psum banks.
  - Epilogue per 512-chunk: s->bf16 copy (DVE), broadcast via ones
    matmul (PE), reciprocal_approx_fast (DVE), o*(1/s) fused into the
    psum->sbuf drain (DVE), po2 = (gamma W_o) @ o_norm (PE),
    out = po2 + x (DVE), DMA per 1024 cols.
  - PSUM exactly 8 banks: pst 2x[128,1024] (4) + psm 4x[*,512] (4)
    shared by convs / po accumulators / pbc / po2 in rotation.
"""

import numpy as np

N_CORES = 8
C = 128
N = 4096       # H*W
M = 1024       # N/4
NCH = 8        # 512-col chunks
CHUNK = 512


def _build(gamma: float, reps: int = 1):
    from contextlib import nullcontext
    import concourse.bass as bass
    import concourse.tile as tile
    from concourse import bacc, mybir

    f32 = mybir.dt.float32
    bf16 = mybir.dt.bfloat16
    ts = bass.ts
    ALU = mybir.AluOpType
    ACTF = mybir.ActivationFunctionType

    nc = bacc.Bacc(
        "TRN2", target_bir_lowering=False, debug=False, enable_asserts=False,
        num_devices=N_CORES,
    )
    xb_d = nc.dram_tensor("xb", [C, N], bf16, kind="ExternalInput")
    # cols 0:64 W_g^T, 64:80 W_phi^T, 80:96 W_theta^T,
    # 96:224 (gamma*W_o)^T on rows 64:128
    wcat_d = nc.dram_tensor("wcat", [128, 224], bf16, kind="ExternalInput")
    id_d = nc.dram_tensor("ident", [64, 64], bf16, kind="ExternalInput")
    out_d = nc.dram_tensor("out", [C, N], f32, kind="ExternalOutput")

    with tile.TileContext(nc) as tc:
        with (
            tc.tile_pool(name="persist", bufs=1) as persist,
            tc.tile_pool(name="et", bufs=10) as etp,
            tc.tile_pool(name="osb", bufs=3) as osbp,
            tc.tile_pool(name="rcb", bufs=3) as rcbp,
            tc.tile_pool(name="ssb", bufs=3) as ssbp,
            tc.tile_pool(name="outp", bufs=3) as outpool,
            tc.tile_pool(name="pst", bufs=2, space="PSUM") as pst,
            tc.tile_pool(name="psm", bufs=4, space="PSUM") as psm,
        ):
          loop_cm = (
              tc.For_i(
                  0, reps, 1,
                  hint_engines=(
                      mybir.EngineType.PE,
                      mybir.EngineType.DVE,
                      mybir.EngineType.Activation,
                      mybir.EngineType.SP,
                      mybir.EngineType.Pool,
                  ),
              )
              if reps > 1
              else nullcontext()
          )
          with loop_cm:
            # ---- loads: wcat on the Act HWDGE queue (parallel with xb on
            # SP); first xb pieces small so conv(0)/conv(1) start early ---
            wcat = persist.tile([128, 224], bf16, name="wcat")
            nc.scalar.dma_start(wcat, wcat_d[:, :])
            xb = persist.tile([C, N], bf16, name="xb")
            for lo, sz in ((0, 512), (512, 512), (1024, 1024),
                           (2048, 1024), (3072, 1024)):
                nc.sync.dma_start(xb[:, bass.ds(lo, sz)],
                                  xb_d[:, bass.ds(lo, sz)])
            id_sb = persist.tile([64, 64], bf16, name="id_sb")
            nc.sync.dma_start(id_sb, id_d[:, :])

            wA = wcat[:, 0:80]          # [W_g^T | W_phi^T]
            wB = wcat[:, 80:96]         # W_theta^T
            wo = wcat[64:128, 96:224]   # (gamma*W_o)^T, K band 64

            ones_sb = persist.tile([1, 128], bf16, name="ones_sb")
            nc.vector.memset(ones_sb, 1.0)

            theta_sb = persist.tile([80, N], bf16, name="theta_sb")  # rows 64:80
            # rows 0:64 g, rows 64:80 phi
            phig_sb = persist.tile([80, M], bf16, name="phig_sb")
            gTa = persist.tile([128, 8 * 128], bf16, name="gTa")
            nc.gpsimd.memset(gTa, 0.0)
            for mi in range(8):
                nc.vector.memset(gTa[:, mi * 128 : mi * 128 + 1], 1.0)

            # ---- per-chunk conv emission -------------------------------
            def conv(ci):
                xc = xb[:, ts(ci, CHUNK)]
                psA = psm.tile([80, CHUNK], f32, name="psA", tag="sm")
                nc.tensor.matmul(psA, wA, xc, start=True, stop=True,
                                 tile_position=(0, 0))
                psB = psm.tile([128, CHUNK], f32, name="psB", tag="sm")
                nc.tensor.matmul(psB[64:80, :], wB, xc, start=True, stop=True,
                                 tile_position=(0, 64))
                # fused g+phi maxpool (DVE) — phi gates S^T(0, ci)
                nc.vector.tensor_reduce(
                    out=phig_sb[:, ts(ci, 128)].rearrange(
                        "p (i j) -> p i j", i=4, j=32),
                    in_=psA.rearrange(
                        "p (i di j dj) -> p i j di dj", i=4, di=2, j=32, dj=2),
                    axis=mybir.AxisListType.XY,
                    op=ALU.max,
                )
                # theta copy: Act during ramp (idle pre-exp; Copy shares the
                # Exp activation table so no reload), DVE after
                dst = theta_sb[64:80, ts(ci, CHUNK)]
                if ci < 2:
                    nc.scalar.copy(dst, psB[64:80, :])
                else:
                    nc.vector.tensor_copy(dst, psB[64:80, :])

            ets = {}

            def emit_st(k, mi):
                st = pst.tile([128, 1024], f32, name="st", tag="st")
                for h in range(2):
                    nc.tensor.matmul(
                        st[:, ts(h, CHUNK)], phig_sb[64:80, ts(mi, 128)],
                        theta_sb[64:80, bass.ds(k * 1024 + h * CHUNK, CHUNK)],
                        start=True, stop=True, tile_position=(64, 0))
                et = etp.tile([128, 1024], bf16, name="et", tag="et")
                nc.scalar.activation(et, st, ACTF.Exp)
                ets[(k, mi)] = et

            def transp(mi):
                ptr = psm.tile([128, 64], bf16, name="ptr", tag="sm")
                nc.tensor.transpose(ptr, phig_sb[0:64, ts(mi, 128)], id_sb)
                nc.vector.tensor_copy(gTa[:, mi * 128 + 64 : mi * 128 + 128],
                                      ptr)

            # ---- attention (v2 schedule: tight S^T/exp/po interleave;
            # the PE stream never queues long runs of blocked matmuls —
            # HW measures ~20% slower when it does, despite the cost
            # model predicting the opposite) ----------------------------
            # per chunk epilogue: s->bf16 (DVE), broadcast via ones matmul
            # (PE), recip full-tile (DVE), o*(1/s) psum drain (DVE),
            # po2 (PE), residual add (DVE), DMA. GPSIMD avoided: ~2us
            # fixed overhead per op on HW.
            def epilogue(k, pos):
                ss, rbs, os = [], [], []
                for c01 in range(2):
                    s_sb = ssbp.tile([1, CHUNK], bf16, name="s_sb", tag="s")
                    nc.vector.tensor_copy(s_sb, pos[c01][0:1, :])
                    ss.append(s_sb)
                # k=3 epilogue psum comes from the pst pool (free after the
                # last exp) so the next iteration's convs are not gated on
                # this iteration's tail via psm slot reuse
                eppool, eptag = (pst, "st") if k == 3 else (psm, "sm")
                pbcs = []
                for c01 in range(2):
                    pbc = eppool.tile([128, CHUNK], f32, name="pbc", tag=eptag)
                    nc.tensor.matmul(pbc, ones_sb, ss[c01], start=True,
                                     stop=True, tile_position=(0, 0))
                    pbcs.append(pbc)
                for c01 in range(2):
                    rbc = rcbp.tile([128, CHUNK], f32, name="rbc", tag="r")
                    nc.vector.reciprocal_approx_fast(rbc, pbcs[c01])
                    rbs.append(rbc)
                for c01 in range(2):
                    o_sb = osbp.tile([128, CHUNK], bf16, name="o_sb", tag="o")
                    nc.vector.tensor_tensor(o_sb[64:128, :],
                                            pos[c01][64:128, :],
                                            rbs[c01][64:128, :], op=ALU.mult)
                    os.append(o_sb)
                po2s = []
                for c01 in range(2):
                    po2 = eppool.tile([128, CHUNK], f32, name="po2", tag=eptag)
                    nc.tensor.matmul(po2, wo, os[c01][64:128, :],
                                     start=True, stop=True,
                                     tile_position=(64, 0))
                    po2s.append(po2)
                outp = outpool.tile([128, 2 * CHUNK], f32, name="outp",
                                    tag="out")
                for c01 in range(2):
                    nc.vector.tensor_tensor(outp[:, ts(c01, CHUNK)],
                                            po2s[c01],
                                            xb[:, ts(2 * k + c01, CHUNK)],
                                            op=ALU.add)
                nc.sync.dma_start(out_d[:, bass.ds(2 * k * CHUNK, 2 * CHUNK)],
                                  outp)

            # k=0 interleaves the convs into the S^T stream (the ramp);
            # transposes batch after conv(7) — they are ready-to-run by
            # then, and the 4 psm slots are fully booked by in-flight
            # convs + the po accumulators, so ptr tiles must not overlap
            # the convs' slots lifetime.
            conv(0)
            conv(1)
            for k in range(4):
                if k == 0:
                    for mi in range(6):
                        emit_st(0, mi)
                        conv(mi + 2)
                    for mi in range(8):
                        transp(mi)

                po_a = psm.tile([128, CHUNK], f32, name="po_a", tag="sm")
                po_b = psm.tile([128, CHUNK], f32, name="po_b", tag="sm")
                pos = (po_a, po_b)

                def po_step(mi):
                    for c01 in range(2):
                        nc.tensor.matmul(
                            pos[c01], gTa[:, ts(mi, 128)],
                            ets[(k, mi)][:, ts(c01, CHUNK)],
                            start=(mi == 0), stop=(mi == 7),
                            tile_position=(0, 0))

                if k == 0:
                    emit_st(0, 6)
                    for mi in range(5):
                        po_step(mi)
                    emit_st(0, 7)
                    po_step(5)
                    po_step(6)
                else:
                    for mi in range(8):
                        emit_st(k, mi)
                        if mi >= 1:
                            po_step(mi - 1)
                po_step(7)
                epilogue(k, pos)

    nc.compile()
    return nc


def _host_prep(x, W_theta, W_phi, W_g, W_o, gamma=0.5):
    import ml_dtypes
    bf16 = ml_dtypes.bfloat16
    x = np.ascontiguousarray(np.asarray(x, dtype=np.float32))
    B = x.shape[0]
    wcat = np.zeros((128, 224), dtype=np.float32)
    wcat[:, 0:64] = np.asarray(W_g, np.float32).T
    wcat[:, 64:80] = np.asarray(W_phi, np.float32).T
    wcat[:, 80:96] = np.asarray(W_theta, np.float32).T
    wcat[64:128, 96:224] = (float(gamma) * np.asarray(W_o, np.float32)).T
    wcat = wcat.astype(bf16)
    ident = np.eye(64, dtype=np.float32).astype(bf16)
    in_maps = []
    for b in range(B):
        xb = np.ascontiguousarray(x[b].reshape(C, N)).astype(bf16)
        in_maps.append({"xb": xb, "wcat": wcat, "ident": ident})
    return in_maps


def run(x, W_theta, W_phi, W_g, W_o, gamma, trace=False, **trace_kwargs):
    from concourse.bass_utils import run_bass_kernel_spmd

    nc = _build(float(np.asarray(gamma)))
    in_maps = _host_prep(x, W_theta, W_phi, W_g, W_o, float(np.asarray(gamma)))
    res = run_bass_kernel_spmd(
        nc, in_maps, core_ids=list(range(N_CORES)), trace=trace, **trace_kwargs
    )
    outs = [res.results[b]["out"].reshape(C, 64, 64) for b in range(N_CORES)]
    return np.stack(outs).astype(np.float32), res


def kernel(x, W_theta, W_phi, W_g, W_o, gamma):
    out, _ = run(x, W_theta, W_phi, W_g, W_o, gamma)
    return out
